# revision 8
# baseline (speedup 1.0000x reference)
"""Causal self-attention Trainium2 kernel.

Problem: B=2, L=2048, D=1024, 16 heads (hd=64), fp32.

Sharding (8 cores): core = (batch b in {0,1}) x (head-group g in {0..3} of 4
heads). Each core:
  - reads x[b]^T  [1024, 2048]
  - QKV projection for its 4 heads (fp32r matmuls, full PE rate at N>=256)
  - causal attention in transposed layout:
      S^T[k, q] = K^T(lhsT) x Q^T(rhs), two heads row-packed per matmul (K=64)
      P^T = exp(S^T)  (ACT, merged tiles over hp x chunk), causal handled at
      128-col granularity: diagonal chunks get restricted matmul widths plus
      one [128,128] triangle-mask multiply (DVE)
      O^T[d, q] accumulated as [V | ones]^T(lhsT) x P^T(rhs); row 64 = rowsum
      normalize: recip = vector.reciprocal of PSUM row 64; partition-broadcast
      via K=1 PE matmul; final multiply on DVE straight out of PSUM
  - output projection partial: OUT[tok, :] = O^T-chunks(lhsT) x Wo^T(rhs),
    PSUM evacuated by GpSimd, DMA out from SBUF
Host: sums the 4 head-group partials per batch, adds out_b + out_w @ bv
(the V-bias contribution commutes through softmax-normalized attention).

All matmul operands are typed float32r (TF32-like, ~1.5e-4 rel err/matmul,
full 1 cycle/row PE rate at N>=256 vs 4 cycles/row for fp32).

Engine balance: PE ~92us is the floor; exp on ACT ~70us; DVE ~45us
(masks/norm/bias); GpSimd takes V and outproj PSUM evacuations (~32us).
"""
import os
import numpy as np

import concourse.bass as bass
import concourse.mybir as mybir
import concourse.tile as tile
from concourse import bacc
from concourse.bass_utils import run_bass_kernel_spmd

F32 = mybir.dt.float32
F32R = mybir.dt.float32r
AF = mybir.ActivationFunctionType

D_MODEL = 1024
N_HEADS = 16
HD = 64
B = 2
L = 2048                      # tokens per batch
HPC = 4                       # heads per core
DG = HPC * HD                 # 256 dims per core's head group
QB = 512                      # q-block width
N_QB = L // QB                # 4
N_DC = D_MODEL // 128         # 8 d_model chunks
N_TT = L // 128               # 16 token tiles


def _build():
    nc = bacc.Bacc("TRN2", target_bir_lowering=False)

    xt = nc.dram_tensor("xt", [N_QB, 128, N_DC, QB], F32R, kind="ExternalInput")
    wq = nc.dram_tensor("wq", [128, N_DC, DG], F32R, kind="ExternalInput")
    wk = nc.dram_tensor("wk", [128, N_DC, DG], F32R, kind="ExternalInput")
    wv = nc.dram_tensor("wv", [128, N_DC, DG], F32R, kind="ExternalInput")
    wo = nc.dram_tensor("wo", [128, 2, D_MODEL], F32R, kind="ExternalInput")
    bq = nc.dram_tensor("bq", [128, 2], F32, kind="ExternalInput")
    bk = nc.dram_tensor("bk", [128, 2], F32, kind="ExternalInput")
    # tri[p, q] = 1 iff q - p >= 0 (on-diagonal 128x128 triangle)
    # ztri[p, q] = 1 iff q - 128 - p >= 0 ([zeros | tri] for half-valid chunk)
    tri = nc.dram_tensor("tri", [128, 128], F32R, kind="ExternalInput")
    ztri = nc.dram_tensor("ztri", [128, 256], F32R, kind="ExternalInput")
    out = nc.dram_tensor("out", [L, D_MODEL], F32, kind="ExternalOutput")

    with tile.TileContext(nc) as tc:
        with (
            tc.tile_pool(name="cst", bufs=1) as cst,
            tc.tile_pool(name="xtp", bufs=2) as xtp,
            tc.tile_pool(name="ptp", bufs=3) as ptp,
            tc.tile_pool(name="nrm", bufs=3) as nrm,
            tc.tile_pool(name="osb", bufs=2) as osb,
            tc.tile_pool(name="ps_st", bufs=1, space="PSUM") as ps_st,
            tc.tile_pool(name="ps_ot", bufs=1, space="PSUM") as ps_ot,
            tc.tile_pool(name="ps_mm", bufs=2, space="PSUM") as ps_mm,
        ):
            # ---- constants / weights ----
            wq_sb = cst.tile([128, N_DC, DG], F32R, tag="wq")
            wk_sb = cst.tile([128, N_DC, DG], F32R, tag="wk")
            wv_sb = cst.tile([128, N_DC, DG], F32R, tag="wv")
            wo_sb = cst.tile([128, 2, D_MODEL], F32R, tag="wo")
            bq_sb = cst.tile([128, 2], F32, tag="bq")
            bk_sb = cst.tile([128, 2], F32, tag="bk")
            tri_sb = cst.tile([128, 128], F32R, tag="tri")
            ztri_sb = cst.tile([128, 256], F32R, tag="ztri")

            ones3_f = cst.tile([128, HPC, HD], F32, tag="ones3_f")
            nc.vector.memset(ones3_f, 1.0)
            ones3 = cst.tile([128, HPC, HD], F32R, tag="ones3")
            nc.vector.tensor_copy(ones3, ones3_f)

            # ---- resident activation tensors ----
            # QT/KT: per head-pair t: [128 (2x64 dims), L]
            qt_sb = [cst.tile([128, L], F32R, tag=f"qt{t}", name=f"qt{t}")
                     for t in range(2)]
            kt_sb = [cst.tile([128, L], F32R, tag=f"kt{t}", name=f"kt{t}")
                     for t in range(2)]
            # OT: per head-pair t: [128 (2x64 dims), L] (normalized)
            ot_sb = [cst.tile([128, L], F32R, tag=f"ot{t}", name=f"ot{t}")
                     for t in range(2)]
            # V natural with 64 replicated ones columns per head (so the
            # O^T matmul emits the softmax denominator replicated on
            # partitions 64..127): per token tile [128 tok, 4 heads, 128]
            v_sb = [cst.tile([128, HPC, 2 * HD], F32R, tag=f"v{tt}",
                             name=f"v{tt}")
                    for tt in range(N_TT)]
            for tt in range(N_TT):
                nc.gpsimd.tensor_copy(v_sb[tt][:, :, HD:], ones3)

            def load_xt(tb, fine=False):
                xt_t = xtp.tile([128, N_DC, QB], F32R, tag="xt", name="xt_t")
                if fine:
                    # pair wq/xt chunks so the first QKV matmuls start ASAP
                    nc.sync.dma_start(xt_t[:, 0:2], xt[tb, :, 0:2])
                    nc.sync.dma_start(wq_sb[:, 2:4], wq[:, 2:4, :])
                    nc.sync.dma_start(xt_t[:, 2:4], xt[tb, :, 2:4])
                    nc.sync.dma_start(xt_t[:, 4:], xt[tb, :, 4:])
                else:
                    nc.sync.dma_start(xt_t[:, 0:N_DC // 2], xt[tb, :, 0:N_DC // 2])
                    nc.sync.dma_start(xt_t[:, N_DC // 2:], xt[tb, :, N_DC // 2:])
                return xt_t

            def qkv_units(tb, xt_t):
                """QKV projection for token block tb as a list of thunks."""
                units = []

                def qk_unit(t, w_sb, b_sb, dst):
                    def f():
                        acc = ps_mm.tile([128, QB], F32, tag="mm", name="acc")
                        for c in range(N_DC):
                            nc.tensor.matmul(
                                acc,
                                w_sb[:, c, 128 * t:128 * (t + 1)],
                                xt_t[:, c, :],
                                start=(c == 0), stop=(c == N_DC - 1),
                            )
                        nc.vector.tensor_scalar_add(
                            dst[t][:, tb * QB:(tb + 1) * QB], acc, b_sb[:, t:t + 1],
                        )
                    return f

                def v_unit(j):
                    def f():
                        tt = tb * (QB // 128) + j
                        vps = ps_mm.tile([128, DG], F32, tag="mm", name="vps")
                        for c in range(N_DC):
                            nc.tensor.matmul(
                                vps,
                                xt_t[:, c, j * 128:(j + 1) * 128],
                                wv_sb[:, c, :],
                                start=(c == 0), stop=(c == N_DC - 1),
                            )
                        nc.vector.tensor_copy(
                            v_sb[tt][:, :, 0:HD],
                            vps.rearrange("p (h d) -> p h d", h=HPC),
                        )
                    return f

                for t in range(2):
                    units.append(qk_unit(t, wq_sb, bq_sb, qt_sb))
                    units.append(qk_unit(t, wk_sb, bk_sb, kt_sb))
                for j in range(QB // 128):
                    units.append(v_unit(j))
                return units

            def attn_units(qb):
                """Attention for q-block qb as a list of thunks.

                k-chunks 0..4qb+3. Off-diagonal groups (2 chunks each) are
                full-width maskless. The 4 diagonal chunks are handled as
                groups A (chunks 4qb,4qb+1) and B (4qb+2,4qb+3) with
                column-restricted matmuls and small triangle masks.
                st tile layout: [128, hp, c, QB] (hp-major, 4 PSUM banks).
                """
                n_kc = 4 * (qb + 1)
                units = []
                ot_state = {}

                def offdiag_unit(t, g):
                    def f():
                        if g == 0:
                            ot_state[t] = {
                                hp: ps_ot.tile([128, QB], F32, tag=f"otp{hp}",
                                               name=f"otp{hp}") for hp in range(2)}
                        ot_p = ot_state[t]
                        st = ps_st.tile([128, 2, 2, QB], F32, tag="st", name="st")
                        for hp in range(2):
                            for c in range(2):
                                kc = 2 * g + c
                                nc.tensor.matmul(
                                    st[:, hp, c, :],
                                    kt_sb[t][64 * hp:64 * (hp + 1),
                                             kc * 128:(kc + 1) * 128],
                                    qt_sb[t][64 * hp:64 * (hp + 1),
                                             qb * QB:(qb + 1) * QB],
                                    start=True, stop=True,
                                )
                        p_t = ptp.tile([128, 2, 2, QB], F32R, tag="pt", name="pt")
                        nc.scalar.activation(p_t, st, AF.Exp)
                        for hp in range(2):
                            for c in range(2):
                                kc = 2 * g + c
                                nc.tensor.matmul(
                                    ot_p[hp],
                                    v_sb[kc][:, 2 * t + hp, :],
                                    p_t[:, hp, c, :],
                                    start=(kc == 0), stop=False,
                                )
                    return f

                def diag_unit(t, half):
                    """half 0: chunks 4qb,4qb+1; half 1: chunks 4qb+2,4qb+3."""
                    def f():
                        if qb == 0 and half == 0:
                            ot_state[t] = {
                                hp: ps_ot.tile([128, QB], F32, tag=f"otp{hp}",
                                               name=f"otp{hp}") for hp in range(2)}
                        ot_p = ot_state[t]
                        st = ps_st.tile([128, 2, 2, QB], F32, tag="st", name="st")
                        kc0 = 4 * qb + 2 * half
                        if half == 0:
                            # c0 full width; c1 valid q >= 128
                            wins = [(0, QB), (128, QB)]
                        else:
                            # both chunks restricted to q >= 256
                            wins = [(256, QB), (256, QB)]
                        for hp in range(2):
                            for c in range(2):
                                q0, q1 = wins[c]
                                nc.tensor.matmul(
                                    st[:, hp, c, q0:q1],
                                    kt_sb[t][64 * hp:64 * (hp + 1),
                                             (kc0 + c) * 128:(kc0 + c + 1) * 128],
                                    qt_sb[t][64 * hp:64 * (hp + 1),
                                             qb * QB + q0:qb * QB + q1],
                                    start=True, stop=True,
                                )
                        p_t = ptp.tile([128, 2, 2, QB], F32R, tag="pt", name="pt")
                        if half == 0:
                            nc.scalar.activation(
                                p_t[:, :, 0, :], st[:, :, 0, :], AF.Exp)
                            nc.scalar.activation(
                                p_t[:, :, 1, 128:], st[:, :, 1, 128:], AF.Exp)
                            # triangle masks on the diagonal 128-col windows
                            for hp in range(2):
                                nc.gpsimd.tensor_mul(
                                    p_t[:, hp, 0, 0:128], p_t[:, hp, 0, 0:128],
                                    tri_sb)
                                nc.gpsimd.tensor_mul(
                                    p_t[:, hp, 1, 128:256],
                                    p_t[:, hp, 1, 128:256], tri_sb)
                        else:
                            nc.scalar.activation(
                                p_t[:, :, :, 256:], st[:, :, :, 256:], AF.Exp)
                            for hp in range(2):
                                nc.gpsimd.tensor_mul(
                                    p_t[:, hp, 0, 256:384],
                                    p_t[:, hp, 0, 256:384], tri_sb)
                                nc.gpsimd.tensor_mul(
                                    p_t[:, hp, 1, 256:], p_t[:, hp, 1, 256:],
                                    ztri_sb)
                        for hp in range(2):
                            for c in range(2):
                                q0, q1 = wins[c]
                                kc = kc0 + c
                                nc.tensor.matmul(
                                    ot_p[hp][:, q0:q1],
                                    v_sb[kc][:, 2 * t + hp, :],
                                    p_t[:, hp, c, q0:q1],
                                    start=(kc == 0), stop=(kc == n_kc - 1),
                                )
                    return f

                def norm_unit(t):
                    def f():
                        ot_p = ot_state[t]
                        for hp in range(2):
                            rcp = nrm.tile([64, QB], F32, tag="rcp",
                                           name="rcp")
                            nc.vector.reciprocal(rcp, ot_p[hp][HD:, :])
                            nc.vector.tensor_mul(
                                ot_sb[t][64 * hp:64 * (hp + 1),
                                         qb * QB:(qb + 1) * QB],
                                ot_p[hp][0:HD, :],
                                rcp,
                            )
                    return f

                for t in range(2):
                    for g in range(2 * qb):
                        units.append(offdiag_unit(t, g))
                    units.append(diag_unit(t, 0))
                    units.append(diag_unit(t, 1))
                    units.append(norm_unit(t))
                return units

            def outproj_units(qb):
                units = []

                def op_unit(j):
                    def f():
                        tt = qb * (QB // 128) + j
                        ob = osb.tile([128, D_MODEL], F32, tag="ob", name="ob")
                        for dc in range(2):
                            ops = ps_mm.tile([128, 512], F32, tag="mm", name="ops")
                            for t in range(2):
                                nc.tensor.matmul(
                                    ops,
                                    ot_sb[t][:, tt * 128:(tt + 1) * 128],
                                    wo_sb[:, t, dc * 512:(dc + 1) * 512],
                                    start=(t == 0), stop=(t == 1),
                                )
                            nc.vector.tensor_copy(
                                ob[:, dc * 512:(dc + 1) * 512], ops)
                        nc.sync.dma_start(out[tt * 128:(tt + 1) * 128, :], ob)
                    return f

                for j in range(QB // 128):
                    units.append(op_unit(j))
                return units

            def emit_interleaved(a_units, b_units):
                """Merge two unit lists proportionally (a is the longer/primary
                stream); keeps relative order within each list."""
                na, nb = len(a_units), len(b_units)
                if nb == 0:
                    for u in a_units:
                        u()
                    return
                bi = 0
                for ai, u in enumerate(a_units):
                    u()
                    want = ((ai + 1) * nb) // na
                    while bi < want:
                        b_units[bi]()
                        bi += 1
                while bi < nb:
                    b_units[bi]()
                    bi += 1

            # ---- emission: interleave attention with QKV/outproj so the PE
            # stays dense while ACT chews the exps ----
            nc.sync.dma_start(wq_sb[:, 0:2], wq[:, 0:2, :])
            nc.sync.dma_start(bq_sb, bq[:, :])
            xt0 = load_xt(0, fine=True)
            nc.sync.dma_start(wq_sb[:, N_DC // 2:], wq[:, N_DC // 2:, :])
            nc.sync.dma_start(wk_sb, wk[:, :, :])
            nc.sync.dma_start(bk_sb, bk[:, :])
            nc.sync.dma_start(wv_sb, wv[:, :, :])
            xt1 = load_xt(1)
            nc.gpsimd.dma_start(tri_sb, tri[:, :])
            nc.gpsimd.dma_start(ztri_sb, ztri[:, :])
            nc.gpsimd.dma_start(wo_sb, wo[:, :, :])

            for u in qkv_units(0, xt0):
                u()
            emit_interleaved(attn_units(0), qkv_units(1, xt1))
            xt2 = load_xt(2)
            xt3 = load_xt(3)
            emit_interleaved(attn_units(1), qkv_units(2, xt2) + outproj_units(0))
            emit_interleaved(attn_units(2), qkv_units(3, xt3) + outproj_units(1))
            emit_interleaved(attn_units(3), outproj_units(2))
            for u in outproj_units(3):
                u()

    nc.compile()
    return nc


_NC_CACHE = None


def _get_nc():
    global _NC_CACHE
    if _NC_CACHE is None:
        _NC_CACHE = _build()
    return _NC_CACHE


def _sw_w(w):
    """[C*128, M] -> [128, C, M] (SBUF layout, contiguous per partition)."""
    c128, m = w.shape
    return np.ascontiguousarray(w.reshape(c128 // 128, 128, m).transpose(1, 0, 2))


def _make_tris():
    p_ = np.arange(128)[:, None]
    q_ = np.arange(128)[None, :]
    tri = (q_ - p_ >= 0).astype(np.float32)
    q2 = np.arange(256)[None, :]
    ztri = (q2 - 128 - p_ >= 0).astype(np.float32)
    return np.ascontiguousarray(tri), np.ascontiguousarray(ztri)


def kernel(x, qkv_w, qkv_b, out_w, out_b, _trace=False):
    x = np.asarray(x, dtype=np.float32)
    qkv_w = np.asarray(qkv_w, dtype=np.float32)
    qkv_b = np.asarray(qkv_b, dtype=np.float32)
    out_w = np.asarray(out_w, dtype=np.float32)
    out_b = np.asarray(out_b, dtype=np.float32)

    scale = 1.0 / np.sqrt(HD)
    wq_full = qkv_w[0:D_MODEL] * scale          # [1024, 1024]
    wk_full = qkv_w[D_MODEL:2 * D_MODEL]
    wv_full = qkv_w[2 * D_MODEL:3 * D_MODEL]
    bq_full = qkv_b[0:D_MODEL] * scale
    bk_full = qkv_b[D_MODEL:2 * D_MODEL]
    bv_full = qkv_b[2 * D_MODEL:3 * D_MODEL]

    tri, ztri = _make_tris()
    in_maps = []
    for core in range(8):
        b, g = core // 4, core % 4
        sl = slice(DG * g, DG * (g + 1))
        # xt: x[b]^T [1024, 2048] -> [qb, p, c, t] = [4, 128, 8, 512]
        xt_sw = np.ascontiguousarray(
            x[b].T.reshape(N_DC, 128, N_QB, QB).transpose(2, 1, 0, 3))
        in_maps.append({
            "xt": xt_sw,
            "wq": _sw_w(wq_full[sl].T),
            "wk": _sw_w(wk_full[sl].T),
            "wv": _sw_w(wv_full[sl].T),
            "wo": _sw_w(out_w[:, sl].T),
            "bq": np.ascontiguousarray(bq_full[sl].reshape(2, 128).T),
            "bk": np.ascontiguousarray(bk_full[sl].reshape(2, 128).T),
            "tri": tri,
            "ztri": ztri,
        })

    nc = _get_nc()
    res = run_bass_kernel_spmd(nc, in_maps, core_ids=list(range(8)),
                               trace=_trace)

    final = np.zeros((B, L, D_MODEL), dtype=np.float32)
    for core in range(8):
        b = core // 4
        final[b] += res.results[core]["out"]
    # out_b plus the V-bias contribution (softmax rows sum to 1, so the
    # bv term passes through attention unchanged and picks up out_w)
    final += (out_b + out_w @ bv_full)[None, None, :]

    kernel.last_results = res
    return final


# revision 10
# speedup vs baseline: 1.6059x; 1.6059x over previous
"""Causal self-attention Trainium2 kernel.

Problem: B=2, L=2048, D=1024, 16 heads (hd=64), fp32.

Sharding (8 cores): core = (batch b in {0,1}) x (head-group g in {0..3} of 4
heads). Each core:
  - reads x[b]^T  [1024, 2048]
  - QKV projection for its 4 heads (fp32r matmuls, full PE rate at N>=256)
  - causal attention in transposed layout:
      S^T[k, q] = K^T(lhsT) x Q^T(rhs), two heads row-packed per matmul (K=64)
      P^T = exp(S^T)  (ACT, merged tiles over hp x chunk), causal handled at
      128-col granularity: diagonal chunks get restricted matmul widths plus
      one [128,128] triangle-mask multiply (DVE)
      O^T[d, q] accumulated as [V | ones]^T(lhsT) x P^T(rhs); row 64 = rowsum
      normalize: recip = vector.reciprocal of PSUM row 64; partition-broadcast
      via K=1 PE matmul; final multiply on DVE straight out of PSUM
  - output projection partial: OUT[tok, :] = O^T-chunks(lhsT) x Wo^T(rhs),
    PSUM evacuated by GpSimd, DMA out from SBUF
Host: sums the 4 head-group partials per batch, adds out_b + out_w @ bv
(the V-bias contribution commutes through softmax-normalized attention).

All matmul operands are typed float32r (TF32-like, ~1.5e-4 rel err/matmul,
full 1 cycle/row PE rate at N>=256 vs 4 cycles/row for fp32).

Engine balance: PE ~92us is the floor; exp on ACT ~70us; DVE ~45us
(masks/norm/bias); GpSimd takes V and outproj PSUM evacuations (~32us).
"""
import os
import numpy as np

import concourse.bass as bass
import concourse.mybir as mybir
import concourse.tile as tile
from concourse import bacc
from concourse.bass_utils import run_bass_kernel_spmd

F32 = mybir.dt.float32
F32R = mybir.dt.float32r
AF = mybir.ActivationFunctionType

D_MODEL = 1024
N_HEADS = 16
HD = 64
B = 2
L = 2048                      # tokens per batch
HPC = 4                       # heads per core
DG = HPC * HD                 # 256 dims per core's head group
QB = 512                      # q-block width
N_QB = L // QB                # 4
N_DC = D_MODEL // 128         # 8 d_model chunks
N_TT = L // 128               # 16 token tiles


def _build():
    nc = bacc.Bacc("TRN2", target_bir_lowering=False)

    xt = nc.dram_tensor("xt", [N_QB, 128, N_DC, QB], F32R, kind="ExternalInput")
    wq = nc.dram_tensor("wq", [128, N_DC, DG], F32R, kind="ExternalInput")
    wk = nc.dram_tensor("wk", [128, N_DC, DG], F32R, kind="ExternalInput")
    wv = nc.dram_tensor("wv", [128, N_DC, DG], F32R, kind="ExternalInput")
    wo = nc.dram_tensor("wo", [128, 2, D_MODEL], F32R, kind="ExternalInput")
    bq = nc.dram_tensor("bq", [128, 2], F32, kind="ExternalInput")
    bk = nc.dram_tensor("bk", [128, 2], F32, kind="ExternalInput")
    # tri[p, q] = 1 iff q - p >= 0 (on-diagonal 128x128 triangle)
    # ztri[p, q] = 1 iff q - 128 - p >= 0 ([zeros | tri] for half-valid chunk)
    tri = nc.dram_tensor("tri", [128, 128], F32R, kind="ExternalInput")
    ztri = nc.dram_tensor("ztri", [128, 256], F32R, kind="ExternalInput")
    out = nc.dram_tensor("out", [L, D_MODEL], F32, kind="ExternalOutput")

    with tile.TileContext(nc) as tc:
        with (
            tc.tile_pool(name="cst", bufs=1) as cst,
            tc.tile_pool(name="xtp", bufs=2) as xtp,
            tc.tile_pool(name="ptp", bufs=3) as ptp,
            tc.tile_pool(name="nrm", bufs=3) as nrm,
            tc.tile_pool(name="osb", bufs=2) as osb,
            tc.tile_pool(name="ps_st", bufs=1, space="PSUM") as ps_st,
            tc.tile_pool(name="ps_ot", bufs=1, space="PSUM") as ps_ot,
            tc.tile_pool(name="ps_mm", bufs=2, space="PSUM") as ps_mm,
        ):
            # ---- constants / weights ----
            wq_sb = cst.tile([128, N_DC, DG], F32R, tag="wq")
            wk_sb = cst.tile([128, N_DC, DG], F32R, tag="wk")
            wv_sb = cst.tile([128, N_DC, DG], F32R, tag="wv")
            wo_sb = cst.tile([128, 2, D_MODEL], F32R, tag="wo")
            bq_sb = cst.tile([128, 2], F32, tag="bq")
            bk_sb = cst.tile([128, 2], F32, tag="bk")
            tri_sb = cst.tile([128, 128], F32R, tag="tri")
            ztri_sb = cst.tile([128, 256], F32R, tag="ztri")

            ones3_f = cst.tile([128, HPC, HD], F32, tag="ones3_f")
            nc.vector.memset(ones3_f, 1.0)
            ones3 = cst.tile([128, HPC, HD], F32R, tag="ones3")
            nc.vector.tensor_copy(ones3, ones3_f)

            # ---- resident activation tensors ----
            # QT/KT: per head-pair t: [128 (2x64 dims), L]
            qt_sb = [cst.tile([128, L], F32R, tag=f"qt{t}", name=f"qt{t}")
                     for t in range(2)]
            kt_sb = [cst.tile([128, L], F32R, tag=f"kt{t}", name=f"kt{t}")
                     for t in range(2)]
            # OT: per head-pair t: [128 (2x64 dims), L] (normalized)
            ot_sb = [cst.tile([128, L], F32R, tag=f"ot{t}", name=f"ot{t}")
                     for t in range(2)]
            # V natural with 64 replicated ones columns per head (so the
            # O^T matmul emits the softmax denominator replicated on
            # partitions 64..127): per token tile [128 tok, 4 heads, 128]
            v_sb = [cst.tile([128, HPC, 2 * HD], F32R, tag=f"v{tt}",
                             name=f"v{tt}")
                    for tt in range(N_TT)]
            for tt in range(N_TT):
                nc.gpsimd.tensor_copy(v_sb[tt][:, :, HD:], ones3)

            def load_xt(tb, fine=False):
                xt_t = xtp.tile([128, N_DC, QB], F32R, tag="xt", name="xt_t")
                if fine:
                    # pair wq/xt chunks so the first QKV matmuls start ASAP
                    nc.sync.dma_start(xt_t[:, 0:2], xt[tb, :, 0:2])
                    nc.sync.dma_start(wq_sb[:, 2:4], wq[:, 2:4, :])
                    nc.sync.dma_start(xt_t[:, 2:4], xt[tb, :, 2:4])
                    nc.sync.dma_start(xt_t[:, 4:], xt[tb, :, 4:])
                else:
                    nc.sync.dma_start(xt_t[:, 0:N_DC // 2], xt[tb, :, 0:N_DC // 2])
                    nc.sync.dma_start(xt_t[:, N_DC // 2:], xt[tb, :, N_DC // 2:])
                return xt_t

            def qkv_units(tb, xt_t):
                """QKV projection for token block tb as a list of thunks."""
                units = []

                def qk_unit(t, w_sb, b_sb, dst):
                    def f():
                        acc = ps_mm.tile([128, QB], F32, tag="mm", name="acc")
                        for c in range(N_DC):
                            nc.tensor.matmul(
                                acc,
                                w_sb[:, c, 128 * t:128 * (t + 1)],
                                xt_t[:, c, :],
                                start=(c == 0), stop=(c == N_DC - 1),
                            )
                        nc.vector.tensor_scalar_add(
                            dst[t][:, tb * QB:(tb + 1) * QB], acc, b_sb[:, t:t + 1],
                        )
                    return f

                def v_unit(j):
                    def f():
                        tt = tb * (QB // 128) + j
                        vps = ps_mm.tile([128, DG], F32, tag="mm", name="vps")
                        for c in range(N_DC):
                            nc.tensor.matmul(
                                vps,
                                xt_t[:, c, j * 128:(j + 1) * 128],
                                wv_sb[:, c, :],
                                start=(c == 0), stop=(c == N_DC - 1),
                            )
                        nc.vector.tensor_copy(
                            v_sb[tt][:, :, 0:HD],
                            vps.rearrange("p (h d) -> p h d", h=HPC),
                        )
                    return f

                for t in range(2):
                    units.append(qk_unit(t, wq_sb, bq_sb, qt_sb))
                    units.append(qk_unit(t, wk_sb, bk_sb, kt_sb))
                for j in range(QB // 128):
                    units.append(v_unit(j))
                return units

            def attn_units(qb):
                """Attention for q-block qb as a list of thunks.

                k-chunks 0..4qb+3. Off-diagonal groups (2 chunks each) are
                full-width maskless. The 4 diagonal chunks are handled as
                groups A (chunks 4qb,4qb+1) and B (4qb+2,4qb+3) with
                column-restricted matmuls and small triangle masks (gpsimd).
                Per-hp st tiles ([128, c, QB], 2 banks) keep the S^T ->
                exp -> O^T chain double-buffered across hp.
                """
                n_kc = 4 * (qb + 1)
                units = []
                ot_state = {}

                def alloc_ot(t):
                    ot_state[t] = {
                        hp: ps_ot.tile([128, QB], F32, tag=f"otp{hp}",
                                       name=f"otp{hp}") for hp in range(2)}

                def offdiag_unit(t, g):
                    def f():
                        if g == 0:
                            alloc_ot(t)
                        ot_p = ot_state[t]
                        st = [ps_st.tile([128, 2, QB], F32, tag=f"st{hp}",
                                         name=f"st{hp}") for hp in range(2)]
                        for hp in range(2):
                            for c in range(2):
                                kc = 2 * g + c
                                nc.tensor.matmul(
                                    st[hp][:, c, :],
                                    kt_sb[t][64 * hp:64 * (hp + 1),
                                             kc * 128:(kc + 1) * 128],
                                    qt_sb[t][64 * hp:64 * (hp + 1),
                                             qb * QB:(qb + 1) * QB],
                                    start=True, stop=True,
                                )
                        for hp in range(2):
                            p_t = ptp.tile([128, 2, QB], F32R, tag=f"pt{hp}",
                                           name=f"pt{hp}")
                            nc.scalar.activation(p_t, st[hp], AF.Exp)
                            for c in range(2):
                                kc = 2 * g + c
                                nc.tensor.matmul(
                                    ot_p[hp],
                                    v_sb[kc][:, 2 * t + hp, :],
                                    p_t[:, c, :],
                                    start=(kc == 0), stop=False,
                                )
                    return f

                def diag_unit(t, half):
                    """half 0: chunks 4qb,4qb+1; half 1: chunks 4qb+2,4qb+3."""
                    def f():
                        if qb == 0 and half == 0:
                            alloc_ot(t)
                        ot_p = ot_state[t]
                        st = [ps_st.tile([128, 2, QB], F32, tag=f"st{hp}",
                                         name=f"st{hp}") for hp in range(2)]
                        kc0 = 4 * qb + 2 * half
                        wins = [(0, QB), (128, QB)] if half == 0 else \
                               [(256, QB), (256, QB)]
                        for hp in range(2):
                            for c in range(2):
                                q0, q1 = wins[c]
                                nc.tensor.matmul(
                                    st[hp][:, c, q0:q1],
                                    kt_sb[t][64 * hp:64 * (hp + 1),
                                             (kc0 + c) * 128:(kc0 + c + 1) * 128],
                                    qt_sb[t][64 * hp:64 * (hp + 1),
                                             qb * QB + q0:qb * QB + q1],
                                    start=True, stop=True,
                                )
                        for hp in range(2):
                            p_t = ptp.tile([128, 2, QB], F32R, tag=f"pt{hp}",
                                           name=f"pt{hp}")
                            if half == 0:
                                nc.scalar.activation(
                                    p_t[:, 0, :], st[hp][:, 0, :], AF.Exp)
                                nc.gpsimd.tensor_mul(
                                    p_t[:, 0, 0:128], p_t[:, 0, 0:128], tri_sb)
                                nc.scalar.activation(
                                    p_t[:, 1, 128:], st[hp][:, 1, 128:], AF.Exp)
                                nc.gpsimd.tensor_mul(
                                    p_t[:, 1, 128:256], p_t[:, 1, 128:256],
                                    tri_sb)
                            else:
                                nc.scalar.activation(
                                    p_t[:, :, 256:], st[hp][:, :, 256:], AF.Exp)
                                nc.gpsimd.tensor_mul(
                                    p_t[:, 0, 256:384], p_t[:, 0, 256:384],
                                    tri_sb)
                                nc.gpsimd.tensor_mul(
                                    p_t[:, 1, 256:], p_t[:, 1, 256:], ztri_sb)
                            for c in range(2):
                                q0, q1 = wins[c]
                                kc = kc0 + c
                                nc.tensor.matmul(
                                    ot_p[hp][:, q0:q1],
                                    v_sb[kc][:, 2 * t + hp, :],
                                    p_t[:, c, q0:q1],
                                    start=(kc == 0), stop=(kc == n_kc - 1),
                                )
                    return f

                def norm_unit(t):
                    def f():
                        ot_p = ot_state[t]
                        for hp in range(2):
                            rs = nrm.tile([64, QB], F32, tag="rs", name="rs")
                            nc.vector.tensor_copy(rs, ot_p[hp][HD:, :])
                            rcp = nrm.tile([64, QB], F32, tag="rcp",
                                           name="rcp")
                            nc.vector.reciprocal_approx_fast(rcp, rs)
                            nc.vector.tensor_mul(
                                ot_sb[t][64 * hp:64 * (hp + 1),
                                         qb * QB:(qb + 1) * QB],
                                ot_p[hp][0:HD, :],
                                rcp,
                            )
                    return f

                for t in range(2):
                    for g in range(2 * qb):
                        units.append(offdiag_unit(t, g))
                    units.append(diag_unit(t, 0))
                    units.append(diag_unit(t, 1))
                    units.append(norm_unit(t))
                return units

            def outproj_units(qb):
                units = []

                def op_unit(j):
                    def f():
                        tt = qb * (QB // 128) + j
                        ob = osb.tile([128, D_MODEL], F32, tag="ob", name="ob")
                        for dc in range(2):
                            ops = ps_mm.tile([128, 512], F32, tag="mm", name="ops")
                            for t in range(2):
                                nc.tensor.matmul(
                                    ops,
                                    ot_sb[t][:, tt * 128:(tt + 1) * 128],
                                    wo_sb[:, t, dc * 512:(dc + 1) * 512],
                                    start=(t == 0), stop=(t == 1),
                                )
                            nc.vector.tensor_copy(
                                ob[:, dc * 512:(dc + 1) * 512], ops)
                        nc.sync.dma_start(out[tt * 128:(tt + 1) * 128, :], ob)
                    return f

                for j in range(QB // 128):
                    units.append(op_unit(j))
                return units

            def emit_interleaved(a_units, b_units):
                """Merge two unit lists proportionally (a is the longer/primary
                stream); keeps relative order within each list."""
                na, nb = len(a_units), len(b_units)
                if nb == 0:
                    for u in a_units:
                        u()
                    return
                bi = 0
                for ai, u in enumerate(a_units):
                    u()
                    want = ((ai + 1) * nb) // na
                    while bi < want:
                        b_units[bi]()
                        bi += 1
                while bi < nb:
                    b_units[bi]()
                    bi += 1

            # ---- emission: interleave attention with QKV/outproj so the PE
            # stays dense while ACT chews the exps ----
            nc.sync.dma_start(wq_sb[:, 0:2], wq[:, 0:2, :])
            nc.sync.dma_start(bq_sb, bq[:, :])
            xt0 = load_xt(0, fine=True)
            nc.sync.dma_start(wq_sb[:, N_DC // 2:], wq[:, N_DC // 2:, :])
            nc.sync.dma_start(wk_sb, wk[:, :, :])
            nc.sync.dma_start(bk_sb, bk[:, :])
            nc.sync.dma_start(wv_sb, wv[:, :, :])
            xt1 = load_xt(1)
            nc.gpsimd.dma_start(tri_sb, tri[:, :])
            nc.gpsimd.dma_start(ztri_sb, ztri[:, :])
            nc.gpsimd.dma_start(wo_sb, wo[:, :, :])

            for u in qkv_units(0, xt0):
                u()
            emit_interleaved(attn_units(0), qkv_units(1, xt1))
            xt2 = load_xt(2)
            xt3 = load_xt(3)
            emit_interleaved(attn_units(1), qkv_units(2, xt2) + outproj_units(0))
            emit_interleaved(attn_units(2), qkv_units(3, xt3) + outproj_units(1))
            emit_interleaved(attn_units(3), outproj_units(2))
            for u in outproj_units(3):
                u()

    nc.compile()
    return nc


_NC_CACHE = None


def _get_nc():
    global _NC_CACHE
    if _NC_CACHE is None:
        _NC_CACHE = _build()
    return _NC_CACHE


def _sw_w(w):
    """[C*128, M] -> [128, C, M] (SBUF layout, contiguous per partition)."""
    c128, m = w.shape
    return np.ascontiguousarray(w.reshape(c128 // 128, 128, m).transpose(1, 0, 2))


def _make_tris():
    p_ = np.arange(128)[:, None]
    q_ = np.arange(128)[None, :]
    tri = (q_ - p_ >= 0).astype(np.float32)
    q2 = np.arange(256)[None, :]
    ztri = (q2 - 128 - p_ >= 0).astype(np.float32)
    return np.ascontiguousarray(tri), np.ascontiguousarray(ztri)


def kernel(x, qkv_w, qkv_b, out_w, out_b, _trace=False):
    x = np.asarray(x, dtype=np.float32)
    qkv_w = np.asarray(qkv_w, dtype=np.float32)
    qkv_b = np.asarray(qkv_b, dtype=np.float32)
    out_w = np.asarray(out_w, dtype=np.float32)
    out_b = np.asarray(out_b, dtype=np.float32)

    scale = 1.0 / np.sqrt(HD)
    wq_full = qkv_w[0:D_MODEL] * scale          # [1024, 1024]
    wk_full = qkv_w[D_MODEL:2 * D_MODEL]
    wv_full = qkv_w[2 * D_MODEL:3 * D_MODEL]
    bq_full = qkv_b[0:D_MODEL] * scale
    bk_full = qkv_b[D_MODEL:2 * D_MODEL]
    bv_full = qkv_b[2 * D_MODEL:3 * D_MODEL]

    tri, ztri = _make_tris()
    in_maps = []
    for core in range(8):
        b, g = core // 4, core % 4
        sl = slice(DG * g, DG * (g + 1))
        # xt: x[b]^T [1024, 2048] -> [qb, p, c, t] = [4, 128, 8, 512]
        xt_sw = np.ascontiguousarray(
            x[b].T.reshape(N_DC, 128, N_QB, QB).transpose(2, 1, 0, 3))
        in_maps.append({
            "xt": xt_sw,
            "wq": _sw_w(wq_full[sl].T),
            "wk": _sw_w(wk_full[sl].T),
            "wv": _sw_w(wv_full[sl].T),
            "wo": _sw_w(out_w[:, sl].T),
            "bq": np.ascontiguousarray(bq_full[sl].reshape(2, 128).T),
            "bk": np.ascontiguousarray(bk_full[sl].reshape(2, 128).T),
            "tri": tri,
            "ztri": ztri,
        })

    nc = _get_nc()
    res = run_bass_kernel_spmd(nc, in_maps, core_ids=list(range(8)),
                               trace=_trace)

    final = np.zeros((B, L, D_MODEL), dtype=np.float32)
    for core in range(8):
        b = core // 4
        final[b] += res.results[core]["out"]
    # out_b plus the V-bias contribution (softmax rows sum to 1, so the
    # bv term passes through attention unchanged and picks up out_w)
    final += (out_b + out_w @ bv_full)[None, None, :]

    kernel.last_results = res
    return final
